# revision 16
# baseline (speedup 1.0000x reference)
"""Trainium2 Bass kernel for nn_DiffusionModel1d (batched 1-D diffusion solve).

Math: the reference solves A(K) u = f per batch row with K = exp(x) via the
Thomas algorithm, where A = G^T diag(K_hat) G, G the n x n lower-bidiagonal
difference matrix (1 on diag, -1 on subdiag) and
K_hat = (2*K_0, K_1, ..., K_{n-1}).  Hence

    u = h2 * G^{-1} diag(K_hat)^{-1} G^{-T} f
      = h2 * cumsum_j( w_j * exp(-x_j) ),   w = suffix_sum(f), w_0 halved.

Layout: TRANSPOSED (grid dim in partitions).  The prefix sum along the grid
dim becomes a per-chunk triangular matrix multiply on the Tensor engine
(lhsT[k, j] = w'_k for k <= j), which removes the Vector-engine scan that
bottlenecked the batch-major version.  Per core (1024 batch cols):

  - 16 grid chunks of 128; e = exp(-x) on ACT (fp16), waves of 2 chunks.
  - main matmul per chunk: local weighted prefix into PSUM fp32.
  - cross-chunk carries: a CAR PSUM tile accumulates SFull_c @ e_c (full
    weight columns -> the chunk total replicated on ALL partitions); one
    fp16 snapshot to SBUF per wave.  Because the carry is replicated, the
    carry add folds into the PSUM->SBUF evacuation as a DVE
    scalar_tensor_tensor (in0*1)+in1 (PSUM fp32 ops are pinned at 1x, so
    the fused add costs the same as a plain copy).  Engines cannot
    partition-broadcast and engine APs must start at partition 0/32/64,
    so a row-per-chunk totals tile is not expressible.  The odd chunk of
    each wave gets the even chunk's contribution as one extra SFull
    matmul accumulated into its PSUM.
  - matmuls are 512 wide (walrus rejects PSUM outputs crossing a bank);
    the carry snapshot is split 768/256 ACT/DVE, balancing both engines,
    and PE warmup matmuls keep the HAM clock gate at 2.4 GHz until the
    weight-gated first real matmul.

DMA plan: x (4 MB) and the wave-ordered weight blocks stream on the sync
HWDGE ring, weights in 4 pieces interleaved after the x pair preceding
their first use, so the first matmul is gated by ~0.8 MB, not the whole
input.  Output stores run on the scalar HWDGE ring so they share the SDMA
queues with the loads at packet granularity instead of queueing behind
them.  Pair 0's x arrives in quarters and its exp runs in quarters so the
first matmul (which only needs e[:, :512]) fires as early as possible.

Everything 16-bit on the wire: x as fp16, weights (per wave-block
[tri_2p | tri_2p+1 | full_2p | full_2p+1], shipped as one [128, 4096]
tensor) fp16 scaled by 2^-4 so they stay in fp16 normal range (h2 * 2^4
applied on host), output fp16.  DMA/core = 4 MB in + 1 MB weights +
4 MB out ~ 9 MB.  Host does the transpose/swizzle so all device DMAs are
contiguous.
"""

import os
import sys

import numpy as np

sys.path.insert(0, "/opt/trn_rl_repo")

import concourse.bacc as bacc
import concourse.mybir as mybir
import concourse.tile as tile
from concourse import bass_utils

B, M = 8192, 2048
N = M - 1
NCORES = 8
BC = B // NCORES          # 1024 batch cols per core
P = 128                   # SBUF partitions
NCH = M // P              # 16 grid chunks per core
NPAIR = NCH // 2          # 8 chunk pairs (one [128, 2048] tile each)
H2 = (1.0 / N) ** 2
SW = 2.0 ** -4            # weight prescale (keeps w' in fp16 normal range)
WB = 4 * P                # weight block cols per wave: tri_e, tri_o, full_e, full_o

_cached_nc = None
LAST_RESULTS = None


def _build_kernel():
    fp32 = mybir.dt.float32
    f16 = mybir.dt.float16
    nc = bacc.Bacc(
        "TRN2",
        target_bir_lowering=False,
        debug=False,
        enable_asserts=False,
        num_devices=NCORES,
    )
    x_d = nc.dram_tensor("x", (BC, 2 * BC), f16, kind="ExternalInput").ap()
    w_d = nc.dram_tensor("w", (P, NPAIR * WB), f16, kind="ExternalInput").ap()
    o_d = nc.dram_tensor("out", (BC, 2 * BC), f16, kind="ExternalOutput").ap()

    EXP = mybir.ActivationFunctionType.Exp
    ADD = mybir.AluOpType.add

    with tile.TileContext(nc) as tc:
        with (
            tc.tile_pool(name="const", bufs=1) as cpool,
            tc.tile_pool(name="xin", bufs=NPAIR) as xpool,
            tc.tile_pool(name="ee", bufs=4) as epool,
            tc.tile_pool(name="oo", bufs=6) as opool,
            tc.tile_pool(name="cs", bufs=3) as cspool,
            tc.tile_pool(name="ps", bufs=3, space="PSUM") as pspool,
            tc.tile_pool(name="pc", bufs=1, space="PSUM") as carpool,
        ):
            # ALL loads on the sync HWDGE ring, in priority order:
            # x0h1, x0h2, wp0, x1, wp1, x2, wp2, x3, wp3, x4..x7.  One
            # ring = strict FIFO arrival, so the first-needed data lands
            # first (two rings share the SDMA queues round-robin at
            # packet granularity, which scrambles arrival order).  The SP
            # sequencer has no compute, so ring-credit backpressure from
            # 4.75 MB of queued loads costs nothing (on the scalar ring
            # it would stall the exps; measured +10 us).  Stores go on
            # the gpsimd SWDGE ring: an independent descriptor path, so
            # they flow during the load stream instead of queueing behind
            # it, and the Pool sequencer can block on evac sems freely.
            xts = []
            for p in range(NPAIR):
                xt = xpool.tile([P, 2 * BC], f16, tag="x")
                xts.append(xt)
            wt = cpool.tile([P, NPAIR * WB], f16, tag="wt")
            WPIECES = 4
            wcols = NPAIR * WB // WPIECES
            nc.sync.dma_start(out=xts[0][:, :BC], in_=x_d[:P, :BC])
            nc.sync.dma_start(out=xts[0][:, BC:], in_=x_d[:P, BC:])
            nc.sync.dma_start(out=wt[:, :wcols], in_=w_d[:, :wcols])
            for p in range(1, NPAIR):
                nc.sync.dma_start(out=xts[p], in_=x_d[p * P : (p + 1) * P, :])
                if p < WPIECES:
                    qs = slice(p * wcols, (p + 1) * wcols)
                    nc.sync.dma_start(out=wt[:, qs], in_=w_d[:, qs])

            # PE warmup: dummy matmuls during the startup dead time so the
            # HAM clock gate is ramped when the first real matmul arrives
            # (~3.5 us in, gated by x quarter 0 + exp + weight piece 0).
            warm = cpool.tile([P, BC], f16, tag="warm")
            nc.gpsimd.memset(warm, 0.0)
            ones1 = cpool.tile([1, P], f16, tag="ones1")
            nc.gpsimd.memset(ones1, 1.0)

            car = carpool.tile([P, BC], fp32, tag="car")
            for _ in range(12):
                nc.tensor.matmul(
                    car[:, :512],
                    lhsT=warm[:, :P],
                    rhs=warm[:, :512],
                    start=True,
                    stop=True,
                    skip_group_check=True,
                )
            carsb = {}

            # exp for wave 0 (pair 0) in halves, then wave 1 up front:
            # the steady-state loop prefetches exp(p+2) during wave p so
            # the PE never waits on the current wave's exp (the
            # cs->exp->PE->car->cs loop otherwise paces the kernel above
            # the engine-busy floor).
            ets = {}
            et0 = epool.tile([P, 2 * BC], f16, tag="e")
            nc.scalar.activation(
                out=et0[:, :BC], in_=xts[0][:, :BC], func=EXP, scale=-1.0
            )
            nc.scalar.activation(
                out=et0[:, BC:], in_=xts[0][:, BC:], func=EXP, scale=-1.0
            )
            ets[0] = et0
            et1 = epool.tile([P, 2 * BC], f16, tag="e")
            nc.scalar.activation(out=et1, in_=xts[1], func=EXP, scale=-1.0)
            ets[1] = et1

            for p in range(NPAIR):  # wave == pair: chunks 2p, 2p+1
                # snapshot the carry through wave p-1, FIRST in this
                # wave's ACT/DVE streams.  The snapshot sits in a 3-way
                # chain: after wave p-1's car matmuls (RAW), before this
                # wave's car matmuls (WAR on the accumulator), and its
                # consumers are this wave's evac-adds.  Issuing it at the
                # top of the body (ACT: cs then exp; PE: car matmuls at
                # the wave END) phase-locks the pipeline -- with the
                # snapshot queued behind the next exp instead, a drift of
                # a few hundred ns grows into a recurring ~1.4 us stall.
                # Split ~70/30 ACT/DVE so neither engine's per-wave load
                # (ACT: exp 1.89, DVE: evac-adds 2.36) exceeds ~2.7.
                if p > 0:
                    cs = cspool.tile([P, BC], f16, tag="cs")
                    nc.scalar.copy(out=cs[:, :768], in_=car[:, :768])
                    nc.vector.scalar_tensor_tensor(
                        out=cs[:, 768:],
                        in0=car[:, 768:],
                        scalar=1.0,
                        in1=warm[:, 768:],
                        op0=mybir.AluOpType.mult,
                        op1=ADD,
                    )
                    carsb[p - 1] = cs
                if p + 2 < NPAIR:   # prefetch exp TWO waves ahead
                    et = epool.tile([P, 2 * BC], f16, tag="e")
                    nc.scalar.activation(
                        out=et, in_=xts[p + 2], func=EXP, scale=-1.0
                    )
                    ets[p + 2] = et
                ep = ets[p]
                wb = p * WB
                ot = opool.tile([P, 2 * BC], f16, tag="o")
                pts = []
                last = p == NPAIR - 1
                for i in range(2):
                    ec = ep[:, i * BC : (i + 1) * BC]
                    tri_c = wt[:, wb + i * P : wb + (i + 1) * P]
                    pt = pspool.tile([P, BC], fp32, tag="ps")
                    pts.append(pt)
                    for h in range(2):
                        hs = slice(h * 512, (h + 1) * 512)
                        nc.tensor.matmul(
                            pt[:, hs],
                            lhsT=tri_c,
                            rhs=ec[:, hs],
                            start=True,
                            stop=(i == 0) and not last,
                        )
                    if i == 1:
                        # even chunk's full contribution into odd chunk
                        full_e = wt[:, wb + 2 * P : wb + 3 * P]
                        for h in range(2):
                            hs = slice(h * 512, (h + 1) * 512)
                            nc.tensor.matmul(
                                pt[:, hs],
                                lhsT=full_e,
                                rhs=ep[:, hs],
                                start=False,
                                stop=not last,
                            )
                    if last:
                        # inject the carry via rank-1 matmuls (TensorE is
                        # idle at the tail) so the final evacs are plain
                        # copies running on ACT and DVE in parallel
                        for h in range(2):
                            hs = slice(h * 512, (h + 1) * 512)
                            nc.tensor.matmul(
                                pt[:, hs],
                                lhsT=ones1,
                                rhs=carsb[p - 1][0:1, hs],
                                start=False,
                                stop=True,
                            )
                # running cross-wave carry accumulator, at the END of the
                # wave's PE stream (gives the WAR with the next snapshot
                # maximum slack on both sides; not needed after the
                # second-to-last wave's snapshot)
                if p + 1 < NPAIR:
                    for i in range(2):
                        c = 2 * p + i
                        ec = ep[:, i * BC : (i + 1) * BC]
                        full_c = wt[:, wb + (2 + i) * P : wb + (3 + i) * P]
                        for h in range(2):
                            hs = slice(h * 512, (h + 1) * 512)
                            nc.tensor.matmul(
                                car[:, hs],
                                lhsT=full_c,
                                rhs=ec[:, hs],
                                start=(c == 0),
                                stop=(c == NCH - 3),
                                skip_group_check=True,
                            )
                # evacuate with fused carry add (replicated on partitions);
                # wave 0 has no carry -> plain DVE copies (ACT is on exps);
                # last wave's carry is already in PSUM -> parallel copies
                for i in range(2):
                    dst = ot[:, i * BC : (i + 1) * BC]
                    if p == 0 or (p == NPAIR - 1 and i == 1):
                        nc.vector.scalar_tensor_tensor(
                            out=dst,
                            in0=pts[i],
                            scalar=1.0,
                            in1=warm,
                            op0=mybir.AluOpType.mult,
                            op1=ADD,
                        )
                    elif p == NPAIR - 1:
                        nc.scalar.copy(out=dst, in_=pts[i])
                    else:
                        # scalar_tensor_tensor computes (in0*1)+in1; the
                        # InstTensorScalarPtr encoding carries a lower
                        # fixed overhead than InstTensorTensor's 151 cyc
                        nc.vector.scalar_tensor_tensor(
                            out=dst,
                            in0=pts[i],
                            scalar=1.0,
                            in1=carsb[p - 1],
                            op0=mybir.AluOpType.mult,
                            op1=ADD,
                        )
                # stores on the sync ring, after all loads in ring order
                # (stores empirically cannot overlap the load stream, and
                # the SWDGE path has ~11 us per-op latency).  Last pair in
                # halves so each half ships as soon as its evac lands.
                if p == NPAIR - 1:
                    nc.sync.dma_start(
                        out=o_d[p * P : (p + 1) * P, :BC], in_=ot[:, :BC]
                    )
                    nc.sync.dma_start(
                        out=o_d[p * P : (p + 1) * P, BC:], in_=ot[:, BC:]
                    )
                else:
                    nc.sync.dma_start(out=o_d[p * P : (p + 1) * P, :], in_=ot)

    nc.compile()
    return nc


def _get_nc():
    global _cached_nc
    if _cached_nc is None:
        _cached_nc = _build_kernel()
    return _cached_nc


def _make_w(f_rhs: np.ndarray) -> np.ndarray:
    """Wave-ordered weights [128, 8*512] fp16.

    Wave block p = [tri_2p | tri_2p+1 | full_2p | full_2p+1], [128, 128]
    each:
      tri_c[k, j]  = w'_{128c+k} * (k <= j)
      full_c[k, j] = w'_{128c+k}
    w' = SW * suffix_sum(f), w'_0 halved, w'_{M-1} = 0 (pad); h2/SW is
    applied on host afterwards.
    """
    w = np.cumsum(f_rhs[::-1].astype(np.float64))[::-1] * SW
    w[0] *= 0.5
    wq = np.zeros(M, np.float16)
    wq[:N] = w.astype(np.float16)
    cols = wq.reshape(NCH, P).T  # [k, c]
    mask = np.arange(P)[:, None] <= np.arange(P)[None, :]
    blocks = []
    for p in range(NPAIR):
        tri_e = cols[:, 2 * p, None] * mask
        tri_o = cols[:, 2 * p + 1, None] * mask
        full_e = np.broadcast_to(cols[:, 2 * p, None], (P, P))
        full_o = np.broadcast_to(cols[:, 2 * p + 1, None], (P, P))
        blocks += [tri_e, tri_o, full_e, full_o]
    return np.ascontiguousarray(
        np.concatenate(blocks, axis=1).astype(np.float16)
    )


def kernel(x: np.ndarray, f_rhs: np.ndarray) -> np.ndarray:
    assert x.shape == (B, M) and f_rhs.shape == (N,)
    wmat = _make_w(np.asarray(f_rhs, dtype=np.float32))
    xf = np.asarray(x, dtype=np.float16)
    in_maps = []
    for c in range(NCORES):
        xt = xf[c * BC : (c + 1) * BC].T  # [M, BC] grid-major
        xs = np.ascontiguousarray(
            xt.reshape(NPAIR, 2, P, BC).transpose(0, 2, 1, 3).reshape(BC, 2 * BC)
        )
        in_maps.append({"x": xs, "w": wmat})
    nc = _get_nc()
    res = bass_utils.run_bass_kernel_spmd(
        nc,
        in_maps,
        core_ids=list(range(NCORES)),
        trace=bool(int(os.environ.get("KERNEL_TRACE", "0"))),
    )
    global LAST_RESULTS
    LAST_RESULTS = res
    outs = []
    post = np.float32(H2 / SW)
    for c in range(NCORES):
        o = res.results[c]["out"]  # [BC, 2*BC] fp16, swizzled u^T
        ut = (
            np.asarray(o)
            .reshape(NPAIR, P, 2, BC)
            .transpose(0, 2, 1, 3)
            .reshape(M, BC)
        )
        outs.append(ut[:N, :].T.astype(np.float32) * post)
    return np.ascontiguousarray(np.concatenate(outs, axis=0))
